# revision 2
# baseline (speedup 1.0000x reference)
"""Trainium2 SPMD kernel for AttentionNodeUpdateNet (GNN message passing).

Strategy (8 NeuronCores, one program, per-core data):
  - Host: sort edges by (dest-node block of 128, direction), pad each
    (block, dir) bucket to a multiple of 128 edges. Core c owns nodes
    [c*6250, (c+1)*6250). Every edge lands on the core that owns its dest
    (row) node, so segment-sum is core-local (no feature all-reduce).
  - Device per core: phase A computes per-node attention projections
    a = xn @ w_att for its node slice; tiny AllGather shares them.
    Pass L gathers a[col]/a[row] per edge (4B indirect DMA), computes
    leaky-relu logits, local masked max/sum-exp; one AllGather of 4
    scalars per core resolves the two global softmax normalizers.
  - Pass E: per 128-edge tile: indirect-gather xn[col] (bf16), DMA-transpose
    edge_attr, per-edge MLP (Linear-LN-ReLU x2) with edges on partitions,
    segment-sum via one-hot matmul accumulated in PSUM per 128-node block.
  - Node phase per block: node update MLP + self MLP from the PSUM
    accumulators (already feature-major), write output slice.
Host reassembles the 8 node slices.
"""

import sys
from contextlib import ExitStack

import numpy as np

sys.path.insert(0, "/opt/trn_rl_repo")

import ml_dtypes  # noqa: E402

import concourse.bass as bass  # noqa: E402
import concourse.bacc as bacc  # noqa: E402
import concourse.tile as tile  # noqa: E402
from concourse import mybir  # noqa: E402
from concourse.bass_utils import run_bass_kernel_spmd  # noqa: E402
from concourse.masks import make_identity  # noqa: E402

BF16 = mybir.dt.bfloat16
F32 = mybir.dt.float32
I32 = mybir.dt.int32
AX = mybir.AxisListType
OP = mybir.AluOpType
AF = mybir.ActivationFunctionType
NPBF = ml_dtypes.bfloat16

N_NODES = 50000
N_EDGES = 400000
D, SPLIT, EA, H = 128, 32, 64, 256
NCORES = 8
NPC = N_NODES // NCORES            # 6250 nodes per core
NB = (NPC + 127) // 128            # 49 node blocks per core
NPAD = NB * 128                    # 6272
NTOT = NCORES * NPAD               # 50176
SLOPE = 0.01
EPS = 1e-5
P = 128
LAST = None  # BassKernelResults of the most recent run (for profiling)


def _prep_host(x, edge_index, edge_attr):
    """Sort/pad edges per (core, block, dir); build per-core device arrays.

    Node blocks of 128 are interleaved across cores (core = block % 8) so the
    direction split (row<col vs row>col) is balanced for every core."""
    xn = np.ascontiguousarray(x[:, :D], dtype=np.float32)
    xsp = np.ascontiguousarray(x[:, D:], dtype=np.float32)
    row = edge_index[0].astype(np.int64)
    col = edge_index[1].astype(np.int64)

    gblk = row >> 7
    core = gblk % NCORES
    k_ = gblk // NCORES                         # block slot within core, 0..NB-1
    rrel = row & 127
    dirm = np.full(N_EDGES, -1, np.int64)
    dirm[row < col] = 0
    dirm[row > col] = 1
    valid = dirm >= 0
    key = (core * NB + k_) * 2 + dirm
    kv = key[valid]
    evi = np.nonzero(valid)[0]
    order = np.argsort(kv, kind="stable")
    skey = kv[order]
    seid = evi[order]
    nbuck = NCORES * NB * 2
    counts = np.bincount(skey, minlength=nbuck).reshape(NCORES, NB, 2)
    # tiles per (block-slot, dir): max over cores, >=1 so PSUM acc is written
    TPBS = np.maximum(1, (counts.max(axis=0) + P - 1) // P)   # [NB, 2]
    # processing order: all dir-0 tiles (k ascending), then all dir-1
    tb = np.zeros((2, NB), np.int64)
    tot = 0
    for d in range(2):
        for k in range(NB):
            tb[d, k] = tot
            tot += TPBS[k, d]
    T = int(tot)
    T0 = int(tb[1, 0])
    EPC = T * P

    starts = np.zeros(nbuck + 1, np.int64)
    np.cumsum(counts.reshape(-1), out=starts[1:])
    rank = np.arange(len(skey)) - starts[skey]
    kk = (skey // 2) % NB
    dd = skey % 2
    slot_in_core = tb[dd, kk] * P + rank
    ecore = skey // (NB * 2)

    xnb = xn.astype(NPBF)
    cidx = np.zeros((NCORES, EPC), np.int32)
    gidx = np.zeros((NCORES, EPC, 2), np.int32)
    rrelp = np.full((NCORES, EPC), 999.0, np.float32)
    padm = np.zeros((NCORES, EPC), np.float32)
    eap = np.zeros((NCORES, EPC, EA), NPBF)

    pid = ecore * EPC + slot_in_core
    cidx.reshape(-1)[pid] = col[seid].astype(np.int32)
    cblk = col[seid] >> 7
    ac_idx = 2 * ((cblk % NCORES) * NPAD + (cblk // NCORES) * P + (col[seid] & 127))
    ar_idx = 2 * (core[seid] * NPAD + k_[seid] * P + rrel[seid]) + 1
    gflat = gidx.reshape(-1)
    gflat[2 * pid] = ac_idx.astype(np.int32)
    gflat[2 * pid + 1] = ar_idx.astype(np.int32)
    rrelp.reshape(-1)[pid] = rrel[seid].astype(np.float32)
    padm.reshape(-1)[pid] = 1.0
    eap.reshape(-1, EA)[pid] = edge_attr[seid].astype(NPBF)

    def colmaj(a, w):
        return np.ascontiguousarray(
            a.reshape(T, P, w).transpose(1, 0, 2).reshape(P, T * w)
        )

    # node rows of core c: global node ids for local row k*128+i -> (8k+c)*128+i
    loc = np.arange(NPAD)
    shards = []
    for c in range(NCORES):
        gid = ((loc // P) * NCORES + c) * P + (loc % P)
        ok = gid < N_NODES
        gc = np.clip(gid, 0, N_NODES - 1)
        xsl = np.where(ok[:, None], xnb[gc], NPBF(0))
        xspl = np.where(ok[:, None], xsp[gc], 0.0).astype(np.float32)
        shards.append(
            dict(
                xg=xnb,
                xsl=np.ascontiguousarray(xsl),
                xsp=np.ascontiguousarray(xspl),
                eatt=np.ascontiguousarray(eap[c].T),
                cidx=colmaj(cidx[c].reshape(EPC, 1), 1).astype(np.int32),
                gidx=colmaj(gidx[c], 2).astype(np.int32),
                rrel=colmaj(rrelp[c].reshape(EPC, 1), 1),
                padm=colmaj(padm[c].reshape(EPC, 1), 1),
            )
        )
    return shards, TPBS, T, T0, EPC


def assemble_output(outs):
    """Interleaved per-core [NPAD,160] slices -> full [50000,160]."""
    full = np.zeros((NCORES * NPAD, D + SPLIT), np.float32)
    loc = np.arange(NPAD)
    for c in range(NCORES):
        gid = ((loc // P) * NCORES + c) * P + (loc % P)
        full[gid] = outs[c]
    return full[:N_NODES]


def _pack_params(inp):
    """Broadcast-tile f32 blob + bf16 matmul-weight blob (both replicated)."""

    def bc(v):  # [k] -> [128, k] broadcast
        v = np.asarray(v, np.float32).reshape(1, -1)
        return np.broadcast_to(v, (P, v.shape[1]))

    fcols, fmap = [], {}

    def addf(name, v):
        t = bc(v)
        fmap[name] = (sum(c.shape[1] for c in fcols), t.shape[1])
        fcols.append(t)

    g1 = {d: np.asarray(inp[d + "_g1"], np.float32) for d in ("d2t", "t2d")}
    g2 = {d: np.asarray(inp[d + "_g2"], np.float32) for d in ("d2t", "t2d")}
    fold = all((g1[d] > 0).all() and (g2[d] > 0).all() for d in g1)
    for d in ("d2t", "t2d"):
        addf(f"b1_{d}", inp[d + "_b1"])
        addf(f"g1_{d}", inp[d + "_g1"])
        addf(f"be1_{d}", np.asarray(inp[d + "_be1"], np.float32) / g1[d]
             if fold else inp[d + "_be1"])
        addf(f"b2_{d}", inp[d + "_b2"])
        addf(f"g2_{d}", inp[d + "_g2"])
        addf(f"be2_{d}", np.asarray(inp[d + "_be2"], np.float32) / g2[d]
             if fold else inp[d + "_be2"])
    addf("node_b", inp["node_b"])
    addf("node_g", inp["node_g"])
    addf("node_be", inp["node_be"])
    addf("self_b", inp["self_b"])
    addf("self_g", inp["self_g"])
    addf("self_be", inp["self_be"])
    wf = np.ascontiguousarray(np.concatenate(fcols, axis=1), np.float32)

    bcols, bmap = [], {}

    def addb(name, m, rows):
        m = np.asarray(m, np.float32)
        t = np.zeros((P, m.shape[1]), NPBF)
        t[:rows] = m.astype(NPBF)
        bmap[name] = (sum(c.shape[1] for c in bcols), m.shape[1], rows)
        bcols.append(t)

    for d in ("d2t", "t2d"):
        w1 = np.asarray(inp[d + "_w1"], np.float32)
        addb(f"w1a_{d}", w1[:D], D)           # xn part  [128,256]
        addb(f"w1b_{d}", w1[D:], EA)          # ea part  [64,256]
        w2 = np.asarray(inp[d + "_w2"], np.float32)
        if fold:
            w2 = w2 * g1[d][:, None]
        addb(f"w2a_{d}", w2[:P], P)
        addb(f"w2b_{d}", w2[P:], P)
    nw = np.asarray(inp["node_w"], np.float32).copy()
    if fold:
        nw[:P] *= g2["t2d"][:, None]      # flow_total = [t2d | d2t]
        nw[P:] *= g2["d2t"][:, None]
    addb("nwa", nw[:P], P)      # t2d feature rows
    addb("nwb", nw[P:], P)      # d2t feature rows
    addb("sw", inp["self_w"], P)
    watt = np.asarray(inp["w_att"], np.float32).reshape(2 * D)
    addb("watt", np.stack([watt[:D], watt[D:]], axis=1), D)   # [128, 2]
    wb = np.ascontiguousarray(np.concatenate(bcols, axis=1), NPBF)
    return wf, fmap, wb, bmap, fold


def _build(nc, TPBS, T, T0, EPC, b_att, wf_cols, fmap, wb_cols, bmap,
           use_cc=True, fold=False):
    """Emit the Tile program. All shapes/consts identical across cores."""
    NBt = len(TPBS)
    assert NBt == NB

    xg = nc.dram_tensor("xg", [N_NODES, D], BF16, kind="ExternalInput")
    xsl = nc.dram_tensor("xsl", [NPAD, D], BF16, kind="ExternalInput")
    xsp = nc.dram_tensor("xsp", [NPAD, SPLIT], F32, kind="ExternalInput")
    eatt = nc.dram_tensor("eatt", [EA, EPC], BF16, kind="ExternalInput")
    cidx = nc.dram_tensor("cidx", [P, T], I32, kind="ExternalInput")
    gidx = nc.dram_tensor("gidx", [P, 2 * T], I32, kind="ExternalInput")
    rrel = nc.dram_tensor("rrel", [P, T], F32, kind="ExternalInput")
    padm = nc.dram_tensor("padm", [P, T], F32, kind="ExternalInput")
    wfd = nc.dram_tensor("wf", [P, wf_cols], F32, kind="ExternalInput")
    wbd = nc.dram_tensor("wb", [P, wb_cols], BF16, kind="ExternalInput")
    out = nc.dram_tensor("out", [NPAD, D + SPLIT], F32, kind="ExternalOutput")

    a_sl = nc.dram_tensor("a_sl", [NPAD, 2], F32)
    a_all = nc.dram_tensor("a_all", [NTOT, 2], F32, addr_space="Shared")
    mz_in = nc.dram_tensor("mz_in", [1, 4], F32)
    mz_all = nc.dram_tensor("mz_all", [1, 32], F32, addr_space="Shared")

    ctx = ExitStack()
    with ctx:
        tc = ctx.enter_context(tile.TileContext(nc))
        const = ctx.enter_context(tc.tile_pool(name="const", bufs=1))
        sb = ctx.enter_context(tc.tile_pool(name="sb", bufs=4))
        sm = ctx.enter_context(tc.tile_pool(name="sm", bufs=6))
        ps = ctx.enter_context(tc.tile_pool(name="ps", bufs=2, space="PSUM"))
        pstr = ctx.enter_context(tc.tile_pool(name="pstr", bufs=2, space="PSUM"))
        pacc = ctx.enter_context(tc.tile_pool(name="pacc", bufs=1, space="PSUM"))

        # ---- constants ----
        ident_f = const.tile([P, P], F32)
        make_identity(nc, ident_f[:])
        ident_b = const.tile([P, P], BF16)
        make_identity(nc, ident_b[:])
        iota_i = const.tile([P, P], I32)
        nc.gpsimd.iota(iota_i[:], pattern=[[1, P]], base=0, channel_multiplier=0)
        iota_f = const.tile([P, P], F32)
        nc.vector.tensor_copy(iota_f[:], iota_i[:])
        ones_row = const.tile([1, P], F32)
        nc.vector.memset(ones_row[:], 1.0)
        eps_t = const.tile([P, 1], F32)
        nc.vector.memset(eps_t[:], EPS)

        wf_sb = const.tile([P, wf_cols], F32)
        nc.sync.dma_start(wf_sb[:], wfd[:])
        wb_sb = const.tile([P, wb_cols], BF16)
        nc.sync.dma_start(wb_sb[:], wbd[:])

        def WFp(name):
            o, w = fmap[name]
            return wf_sb[:, o:o + w]

        def WBp(name):
            o, w, rows = bmap[name]
            return wb_sb[:rows, o:o + w]

        cidx_sb = const.tile([P, T], I32)
        nc.sync.dma_start(cidx_sb[:], cidx[:])
        gidx_sb = const.tile([P, 2 * T], I32)
        nc.sync.dma_start(gidx_sb[:], gidx[:])
        rrel_sb = const.tile([P, T], F32)
        nc.sync.dma_start(rrel_sb[:], rrel[:])
        padm_sb = const.tile([P, T], F32)
        nc.sync.dma_start(padm_sb[:], padm[:])

        ltab = const.tile([P, T], F32)      # logits
        ltmp = const.tile([P, T], F32)
        atab = const.tile([P, T], F32)      # exp/attention table
        xsT = const.tile([P, NB * P], BF16)  # resident transposed node slices

        # x_split passthrough
        nc.sync.dma_start(out[:, D:D + SPLIT], xsp[:])

        # ---- phase A: a = xn_slice @ [wc|wr], stash xsT ----
        a_stage = const.tile([P, NB * 2], F32)
        for b in range(NB):
            xb = sb.tile([P, D], BF16, tag="xb")
            nc.sync.dma_start(xb[:], xsl[b * P:(b + 1) * P, :])
            pt = pstr.tile([P, P], BF16, tag="tr")
            nc.tensor.transpose(pt[:], xb[:], ident_b[:])
            nc.scalar.copy(xsT[:, b * P:(b + 1) * P], pt[:])
            pa = pstr.tile([P, 2], F32, tag="tr")
            nc.tensor.matmul(pa[:], lhsT=xsT[:, b * P:(b + 1) * P],
                             rhs=WBp("watt"), start=True, stop=True)
            nc.vector.tensor_copy(a_stage[:, 2 * b:2 * b + 2], pa[:])
        # single DMA so the collective waits on one queue only
        nc.sync.dma_start(
            a_sl[:].rearrange("(k p) c -> p k c", p=P),
            a_stage[:].rearrange("p (k c) -> p k c", c=2))

        # ---- collective 1: share per-node projections ----
        if use_cc:
            nc.gpsimd.collective_compute(
                "AllGather", OP.bypass, replica_groups=[list(range(NCORES))],
                ins=[a_sl[:]], outs=[a_all[:]],
            )
        else:  # timing-model stand-in
            nc.sync.dma_start(a_all[0:NPAD, :], a_sl[:])
        a_flat = a_all[:].rearrange("n two -> (n two)")[:, None]

        # ---- pass L: logits (groups of GL tiles) ----
        GL = 16
        for g0 in range(0, T, GL):
            gw = min(GL, T - g0)
            vg = sb.tile([P, 2 * GL], F32, tag="vg")
            nc.gpsimd.indirect_dma_start(
                out=vg[:, :2 * gw], out_offset=None, in_=a_flat,
                in_offset=bass.IndirectOffsetOnAxis(
                    ap=gidx_sb[:, 2 * g0:2 * (g0 + gw)], axis=0),
            )
            v3 = vg[:, :2 * gw].rearrange("p (t two) -> p t two", two=2)
            nc.vector.tensor_tensor(
                out=ltab[:, g0:g0 + gw],
                in0=v3[:, :, 0:1], in1=v3[:, :, 1:2], op=OP.add)
        # leaky_relu(z + b_att) = max(z + b_att, slope*(z + b_att))
        nc.vector.tensor_scalar(out=ltab[:], in0=ltab[:], scalar1=float(b_att),
                                scalar2=None, op0=OP.add)
        nc.vector.tensor_scalar(out=ltmp[:], in0=ltab[:], scalar1=SLOPE,
                                scalar2=None, op0=OP.mult)
        nc.vector.tensor_tensor(out=ltab[:], in0=ltab[:], in1=ltmp[:], op=OP.max)

        # ---- softmax normalizers (global over all cores) ----
        dsl = (slice(0, T0), slice(T0, T))
        mzrow = sm.tile([1, 4], F32, tag="mzrow")
        mps = []
        for d in range(2):
            ld = ltab[:, dsl[d]]
            ad = atab[:, dsl[d]]
            pd = padm_sb[:, dsl[d]]
            mp = sm.tile([P, 1], F32, tag=f"mp{d}")
            mps.append(mp)
            nc.vector.tensor_reduce(mp[:], ld, axis=AX.X, op=OP.max)
            nmp = sm.tile([P, 1], F32, tag="nmp")
            nc.vector.tensor_scalar(out=nmp[:], in0=mp[:], scalar1=-1.0,
                                    scalar2=None, op0=OP.mult)
            nc.scalar.activation(ad, ld, AF.Exp, bias=nmp[:, 0:1], scale=1.0)
            nc.vector.tensor_tensor(out=ad, in0=ad, in1=pd, op=OP.mult)
            zp = sm.tile([P, 1], F32, tag="zp")
            nc.vector.tensor_reduce(zp[:], ad, axis=AX.X, op=OP.add)
            # cross-partition reduction via PE transpose to [1,128]
            pmt = pstr.tile([1, P], F32, tag="tr")
            nc.tensor.transpose(pmt[:], mp[:], ident_f[:])
            mrow = sm.tile([1, P], F32, tag="mrow")
            nc.vector.tensor_copy(mrow[:], pmt[:])
            pzt = pstr.tile([1, P], F32, tag="tr")
            nc.tensor.transpose(pzt[:], zp[:], ident_f[:])
            zrow = sm.tile([1, P], F32, tag="zrow")
            nc.vector.tensor_copy(zrow[:], pzt[:])
            mc = sm.tile([1, 1], F32, tag="mc")
            nc.vector.tensor_reduce(mc[:], mrow[:], axis=AX.X, op=OP.max)
            zs = sm.tile([1, P], F32, tag="zs")
            nc.vector.tensor_scalar(out=zs[:], in0=mrow[:], scalar1=mc[:, 0:1],
                                    scalar2=None, op0=OP.subtract)
            nc.scalar.activation(zs[:], zs[:], AF.Exp)
            nc.vector.tensor_tensor(out=zs[:], in0=zs[:], in1=zrow[:], op=OP.mult)
            zc = sm.tile([1, 1], F32, tag="zc")
            nc.vector.tensor_reduce(zc[:], zs[:], axis=AX.X, op=OP.add)
            nc.vector.tensor_copy(mzrow[:, 2 * d:2 * d + 1], mc[:])
            nc.vector.tensor_copy(mzrow[:, 2 * d + 1:2 * d + 2], zc[:])
        nc.sync.dma_start(mz_in[:], mzrow[:])

        if use_cc:
            nc.gpsimd.collective_compute(
                "AllGather", OP.bypass, replica_groups=[list(range(NCORES))],
                ins=[mz_in[:]], outs=[mz_all[:]],
            )
        else:
            nc.sync.dma_start(mz_all[:, 0:4], mz_in[:])
        mzs = sm.tile([1, 32], F32, tag="mzs")
        nc.sync.dma_start(mzs[:], mz_all[:])
        m3 = mzs[:].rearrange("p (c four) -> p c four", four=4)
        for d in range(2):
            msl = m3[:, :, 2 * d:2 * d + 1]
            zsl = m3[:, :, 2 * d + 1:2 * d + 2]
            mg = sm.tile([1, 1], F32, tag="mg")
            nc.vector.tensor_reduce(mg[:], msl, axis=AX.XY, op=OP.max)
            t8 = sm.tile([1, 8], F32, tag="t8")
            nc.vector.tensor_scalar(out=t8[:], in0=msl, scalar1=mg[:, 0:1],
                                    scalar2=None, op0=OP.subtract)
            nc.scalar.activation(t8[:], t8[:], AF.Exp)
            nc.vector.tensor_tensor(out=t8[:], in0=t8[:], in1=zsl, op=OP.mult)
            zg = sm.tile([1, 1], F32, tag="zg")
            nc.vector.tensor_reduce(zg[:], t8[:], axis=AX.X, op=OP.add)
            nc.scalar.activation(zg[:], zg[:], AF.Ln)
            nbias = sm.tile([1, 1], F32, tag="nbias")
            nc.vector.tensor_tensor(out=nbias[:], in0=mg[:], in1=zg[:], op=OP.add)
            nc.vector.tensor_scalar(out=nbias[:], in0=nbias[:], scalar1=-1.0,
                                    scalar2=None, op0=OP.mult)
            pb = pstr.tile([P, 1], F32, tag="tr")
            nc.tensor.matmul(pb[:], lhsT=ones_row[:], rhs=nbias[:],
                             start=True, stop=True)
            bb = sm.tile([P, 1], F32, tag="bb")
            nc.vector.tensor_copy(bb[:], pb[:])
            # attn = exp(l - mp) * exp(mp - M - lnZ)
            fac = sm.tile([P, 1], F32, tag=f"fac{d}")
            nc.scalar.activation(fac[:], mps[d][:], AF.Exp, bias=bb[:, 0:1],
                                 scale=1.0)
            nc.vector.tensor_scalar(out=atab[:, dsl[d]], in0=atab[:, dsl[d]],
                                    scalar1=fac[:, 0:1], scalar2=None,
                                    op0=OP.mult)

        # ---- pass E + node phase ----
        def ln_apply(src_ap, bias_tile, gt, bet, width, out_dtype, tagp):
            """y = src + bias; u = (y-mean)/sqrt(var+eps); relu(u*g+be)."""
            y = sb.tile([P, width], F32, tag=f"y{tagp}")
            nc.vector.tensor_tensor(out=y[:], in0=src_ap, in1=bias_tile,
                                    op=OP.add)
            bs = sm.tile([P, 6], F32, tag=f"bs{tagp}")
            nc.vector.bn_stats(bs[:], y[:])
            ba = sm.tile([P, 2], F32, tag=f"ba{tagp}")
            nc.vector.bn_aggr(ba[:], bs[:])
            sd = sm.tile([P, 1], F32, tag=f"sd{tagp}")
            nc.scalar.activation(sd[:], ba[:, 1:2], AF.Sqrt, bias=eps_t[:, 0:1],
                                 scale=1.0)
            rs = sm.tile([P, 1], F32, tag=f"rs{tagp}")
            nc.vector.reciprocal(rs[:], sd[:])
            nmr = sm.tile([P, 1], F32, tag=f"nmr{tagp}")
            nc.vector.tensor_scalar(out=nmr[:], in0=ba[:, 0:1],
                                    scalar1=rs[:, 0:1], scalar2=-1.0,
                                    op0=OP.mult, op1=OP.mult)
            u = sb.tile([P, width], F32, tag=f"u{tagp}")
            nc.scalar.activation(u[:], y[:], AF.Identity, bias=nmr[:, 0:1],
                                 scale=rs[:, 0:1])
            if gt is None:   # gamma folded into the next matmul's weights
                nc.gpsimd.tensor_tensor(out=u[:], in0=u[:], in1=bet, op=OP.add)
                v = u
            else:
                v = sb.tile([P, width], F32, tag=f"v{tagp}")
                nc.gpsimd.tensor_tensor(out=v[:], in0=u[:], in1=gt, op=OP.mult)
                nc.vector.tensor_tensor(out=v[:], in0=v[:], in1=bet, op=OP.add)
            hf = sb.tile([P, width], out_dtype, tag=f"hf{tagp}")
            nc.vector.tensor_scalar(out=hf[:], in0=v[:], scalar1=0.0,
                                    scalar2=None, op0=OP.max)
            return hf

        fAC = [const.tile([P, NB * P], BF16, name=f"fAC{d}") for d in range(2)]
        tflat = 0
        for d in range(2):
            dn = ("d2t", "t2d")[d]
            for k in range(NB):
                ntile = int(TPBS[k][d])
                acc = pacc.tile([P, P], F32, tag="acc", name=f"acc{d}_{k}")
                for j in range(ntile):
                    t = tflat
                    tflat += 1
                    xc = sb.tile([P, D], BF16, tag="xc")
                    nc.gpsimd.indirect_dma_start(
                        out=xc[:], out_offset=None, in_=xg[:],
                        in_offset=bass.IndirectOffsetOnAxis(
                            ap=cidx_sb[:, t:t + 1], axis=0),
                    )
                    eaT = sb.tile([EA, P], BF16, tag="eaT")
                    nc.sync.dma_start(eaT[:], eatt[:, t * P:(t + 1) * P])
                    pxc = pstr.tile([P, P], BF16, tag="tr")
                    nc.tensor.transpose(pxc[:], xc[:], ident_b[:])
                    xcT = sb.tile([P, P], BF16, tag="xcT")
                    nc.scalar.copy(xcT[:], pxc[:])

                    ph = ps.tile([P, H], F32, tag="ph")
                    nc.tensor.matmul(ph[:], lhsT=xcT[:], rhs=WBp(f"w1a_{dn}"),
                                     start=True, stop=False)
                    nc.tensor.matmul(ph[:], lhsT=eaT[:], rhs=WBp(f"w1b_{dn}"),
                                     start=False, stop=True)
                    xh = sb.tile([P, H], F32, tag="xh")
                    nc.scalar.activation(xh[:], ph[:], AF.Copy,
                                         scale=atab[:, t:t + 1])
                    h1 = ln_apply(xh[:], WFp(f"b1_{dn}"),
                                  None if fold else WFp(f"g1_{dn}"),
                                  WFp(f"be1_{dn}"), H, BF16, "1")

                    pt1 = pstr.tile([P, P], BF16, tag="tr")
                    nc.tensor.transpose(pt1[:], h1[:, 0:P], ident_b[:])
                    h1a = sb.tile([P, P], BF16, tag="h1a")
                    nc.scalar.copy(h1a[:], pt1[:])
                    pt2 = pstr.tile([P, P], BF16, tag="tr")
                    nc.tensor.transpose(pt2[:], h1[:, P:H], ident_b[:])
                    h1b = sb.tile([P, P], BF16, tag="h1b")
                    nc.scalar.copy(h1b[:], pt2[:])

                    pm2 = ps.tile([P, P], F32, tag="mm")
                    nc.tensor.matmul(pm2[:], lhsT=h1a[:], rhs=WBp(f"w2a_{dn}"),
                                     start=True, stop=False)
                    nc.tensor.matmul(pm2[:], lhsT=h1b[:], rhs=WBp(f"w2b_{dn}"),
                                     start=False, stop=True)
                    hf = ln_apply(pm2[:], WFp(f"b2_{dn}"),
                                  None if fold else WFp(f"g2_{dn}"),
                                  WFp(f"be2_{dn}"), P, BF16, "2")

                    S = sb.tile([P, P], BF16, tag="S")
                    nc.vector.tensor_scalar(
                        out=S[:], in0=iota_f[:], scalar1=rrel_sb[:, t:t + 1],
                        scalar2=None, op0=OP.is_equal)
                    nc.tensor.matmul(acc[:], lhsT=hf[:], rhs=S[:],
                                     start=(j == 0), stop=(j == ntile - 1))
                nc.scalar.copy(fAC[d][:, k * P:(k + 1) * P], acc[:])
        assert tflat == T

        # ---- node phase ----
        for k in range(NB):
            pn = ps.tile([P, P], F32, tag="mm")
            # flow_total = [t2d | d2t] -> node_w rows 0:128 multiply t2d
            nc.tensor.matmul(pn[:], lhsT=fAC[1][:, k * P:(k + 1) * P],
                             rhs=WBp("nwa"), start=True, stop=False)
            nc.tensor.matmul(pn[:], lhsT=fAC[0][:, k * P:(k + 1) * P],
                             rhs=WBp("nwb"), start=False, stop=True)
            un = ln_apply(pn[:], WFp("node_b"), WFp("node_g"), WFp("node_be"),
                          P, F32, "n")
            psf = ps.tile([P, P], F32, tag="mm")
            nc.tensor.matmul(psf[:], lhsT=xsT[:, k * P:(k + 1) * P],
                             rhs=WBp("sw"), start=True, stop=True)
            us = ln_apply(psf[:], WFp("self_b"), WFp("self_g"), WFp("self_be"),
                          P, F32, "s")
            ob = sb.tile([P, P], F32, tag="ob")
            nc.vector.tensor_tensor(out=ob[:], in0=un[:], in1=us[:], op=OP.add)
            nc.sync.dma_start(out[k * P:(k + 1) * P, 0:D], ob[:])

    return nc


def prepare(inputs):
    """Host prep + build + compile. Returns (nc, in_maps, meta)."""
    x = np.asarray(inputs["x"])
    edge_index = np.asarray(inputs["edge_index"])
    edge_attr = np.asarray(inputs["edge_attr"])

    shards, TPBS, T, T0, EPC = _prep_host(x, edge_index, edge_attr)
    wf, fmap, wb, bmap, fold = _pack_params(inputs)

    nc = bacc.Bacc("TRN2", target_bir_lowering=False, debug=False,
                   num_devices=NCORES)
    _build(nc, TPBS, T, T0, EPC,
           float(np.asarray(inputs["b_att"]).reshape(-1)[0]),
           wf.shape[1], fmap, wb.shape[1], bmap, fold=fold)
    nc.compile()

    in_maps = []
    for c in range(NCORES):
        m = dict(shards[c])
        m["wf"] = wf
        m["wb"] = wb
        in_maps.append(m)
    return nc, in_maps, dict(T=T, T0=T0, EPC=EPC)


def kernel(**inputs):
    nc, in_maps, _meta = prepare(inputs)
    res = run_bass_kernel_spmd(nc, in_maps, core_ids=list(range(NCORES)))
    global LAST
    LAST = res
    outs = [np.asarray(res.results[c]["out"], np.float32) for c in range(NCORES)]
    return np.ascontiguousarray(assemble_output(outs))

